# revision 12
# baseline (speedup 1.0000x reference)
"""LocationAwareAttention fused Bass kernel for Trainium2, 8-core data-parallel.

Reference computation (per batch row b):
  loc    = conv1d(last_attn[b], conv_w, pad SAME) + conv_b        # (T, A)
  hidden = tanh(query[b] @ Wq.T + value[b] @ Wv.T + loc + bias)   # (T, A)
  energy = hidden @ fc_w.T (+ fc_b, drops out of softmax)         # (T,)
  attn   = softmax(energy)                                        # (T,)
  ctx    = attn @ value[b]                                        # (D,)

Sharding: data-parallel over batch, 4 batches per core, params replicated.

Device pipeline per core (per 512-wide t-chunk):
  - value loads naturally (t on partitions) with an fp32->fp32r cast DMA,
    then is PE-transposed 128x128 to give vT k-slabs (d on partitions).
  - vproj is computed transposed: psum[a, t] = sum_k WvT[k,a-tile].T @ vT[k],
    with loc+qproj+bias+conv_b folded in as one extra K=6 matmul whose lhsT
    rows are [w0,w1,w2,qb,bias,conv_b] and rhs rows
    [la(t-1),la(t),la(t+1),1,1,1].
  - tanh on ACT -> hidden.T tile; fc contraction over a on the PE
    (lhsT = fc_w chunk (128,1), rhs = hidden.T) accumulating energy (1,512).
  - exp on ACT with accum_out giving the chunk sum; softmax skips
    max-subtraction (|energy| <= sum|fc_w| since |tanh|<=1, safe in fp32).
  - exp row is transposed to partitions via K=1 matmuls; context accumulates
    unnormalized on the PE from the natural value tiles; everything is scaled
    by 1/sum in the batch epilogue.

All matmul operands live in float32r (TF32-like: 1 PE cycle/row vs 4 for
strict fp32 when the moving free dim >= 256); PSUM accumulation is fp32.
"""

import sys
from contextlib import ExitStack

for _p in ("/opt/trn_rl_repo",):
    if _p not in sys.path:
        sys.path.append(_p)

import numpy as np

import concourse.mybir as mybir
import concourse.tile as tile
from concourse import bacc
from concourse.bass_utils import run_bass_kernel_spmd
from concourse.masks import make_identity

F32 = mybir.dt.float32
F32R = mybir.dt.float32r
AF = mybir.ActivationFunctionType

B, T, D, A = 32, 2048, 1024, 1024
NCORES = 8
BL = B // NCORES          # batches per core
P = 128
TCH = 512                 # t-chunk (matmul free dim)
NCC = T // TCH            # chunks per batch = 4
NCH = BL * NCC            # chunks per core = 16
NKT = D // P              # 8 k-tiles over d
NAT = A // P              # 8 a-tiles
NTS = TCH // P            # 4 t-subtiles per chunk


def build_program():
    nc = bacc.Bacc("TRN2", target_bir_lowering=False, debug=False,
                   num_devices=NCORES)

    value = nc.dram_tensor("value", [BL, T, D], F32, kind="ExternalInput")
    qt = nc.dram_tensor("qt", [D, BL], F32, kind="ExternalInput")
    la = nc.dram_tensor("la", [BL, T], F32, kind="ExternalInput")
    wvt = nc.dram_tensor("wvt", [D, A], F32, kind="ExternalInput")
    wqt = nc.dram_tensor("wqt", [D, A], F32, kind="ExternalInput")
    cw3 = nc.dram_tensor("cw3", [3, A], F32, kind="ExternalInput")
    cb = nc.dram_tensor("cb", [1, A], F32, kind="ExternalInput")
    bias_t = nc.dram_tensor("bias", [1, A], F32, kind="ExternalInput")
    fcw = nc.dram_tensor("fcw", [A], F32, kind="ExternalInput")
    ctx_out = nc.dram_tensor("ctx_out", [BL, D], F32, kind="ExternalOutput")
    attn_out = nc.dram_tensor("attn_out", [BL, T], F32, kind="ExternalOutput")

    with tile.TileContext(nc) as tc:
        with ExitStack() as es:
            build_tile_kernel(tc, es, value, qt, la, wvt, wqt, cw3, cb,
                              bias_t, fcw, ctx_out, attn_out)
    nc.compile()
    return nc


def build_tile_kernel(tc, es, value, qt, la, wvt, wqt, cw3, cb, bias_t, fcw,
                      ctx_out, attn_out):
    nc = tc.nc

    persist = es.enter_context(tc.tile_pool(name="persist", bufs=1))
    vnat_pool = es.enter_context(tc.tile_pool(name="vnat", bufs=2))
    vt_pool = es.enter_context(tc.tile_pool(name="vt", bufs=2))
    work = es.enter_context(tc.tile_pool(name="work", bufs=3))
    small = es.enter_context(tc.tile_pool(name="small", bufs=2))
    psum_mm = es.enter_context(
        tc.tile_pool(name="psum_mm", bufs=2, space="PSUM"))
    psum_tp = es.enter_context(
        tc.tile_pool(name="psum_tp", bufs=2, space="PSUM"))
    psum_sm = es.enter_context(
        tc.tile_pool(name="psum_sm", bufs=1, space="PSUM"))
    psum_cx = es.enter_context(
        tc.tile_pool(name="psum_cx", bufs=1, space="PSUM"))
    psum_pe = es.enter_context(
        tc.tile_pool(name="psum_pe", bufs=1, space="PSUM"))

    # ---- persistent parameter tiles (float32r, cast at DMA time) ----
    wvt_sb = persist.tile([P, NKT, A], F32R)
    nc.gpsimd.dma_start(wvt_sb[:], wvt.ap().rearrange("(k p) a -> p k a", p=P))

    ident = persist.tile([P, P], F32)
    make_identity(nc, ident[:])

    fcw_sb = persist.tile([P, NAT], F32)
    nc.sync.dma_start(fcw_sb[:], fcw.ap().rearrange("(j p) -> p j", p=P))

    ones_f = persist.tile([1, 1], F32)
    nc.any.memset(ones_f[:], 1.0)

    zero_sb = persist.tile([1, 1], F32)
    nc.any.memset(zero_sb[:], 0.0)

    # combo lhsT per batch: [6, b, 1024] rows = w0,w1,w2,qb,bias,conv_b
    combo = persist.tile([6, BL, A], F32R)
    for b in range(BL):
        nc.gpsimd.dma_start(combo[0:3, b], cw3.ap())
        nc.gpsimd.dma_start(combo[4:5, b], bias_t.ap())
        nc.gpsimd.dma_start(combo[5:6, b], cb.ap())

    # ---- qproj for all 4 batches: qpn[b, a] = (query @ Wq.T)[b, a] ----
    qt_sb = persist.tile([P, NKT, BL], F32)
    nc.sync.dma_start(qt_sb[:], qt.ap().rearrange("(k p) b -> p k b", p=P))
    qpn = persist.tile([BL, A], F32R)
    for j in range(2):
        psq = psum_mm.tile([P, TCH], F32, tag="mm", name=f"psq{j}")
        for k in range(NKT):
            wq_t = work.tile([P, TCH], F32, tag="wq", name=f"wq{j}_{k}")
            nc.sync.dma_start(
                wq_t[:], wqt.ap()[k * P:(k + 1) * P, j * TCH:(j + 1) * TCH])
            nc.tensor.matmul(psq[:BL], qt_sb[:, k, :], wq_t[:],
                             start=(k == 0), stop=(k == NKT - 1))
        nc.vector.tensor_copy(qpn[:, j * TCH:(j + 1) * TCH], psq[:BL])
    for b in range(BL):
        nc.sync.dma_start(combo[3:4, b], qpn[b:b + 1, :])

    # ---- main loop over the 16 t-chunks ----
    attn_sb = persist.tile([1, BL, T], F32)
    sums = persist.tile([1, BL, NCC], F32)
    pctx = None

    for c in range(NCH):
        b, cc = divmod(c, NCC)
        if cc == 0:
            pctx = psum_cx.tile([1, 2, TCH], F32, tag="ctx", name=f"pctx{b}")
        # 0) la_shift rows for this chunk: [la(t-1);la(t);la(t+1);1;1;1]
        t0 = cc * TCH
        la_c = work.tile([6, TCH], F32, tag="laf", name=f"laf{c}")
        nc.any.memset(la_c[:], 1.0)
        for r in range(3):
            lo = t0 - 1 + r
            hi = lo + TCH
            clo, chi = max(lo, 0), min(hi, T)
            off = clo - lo
            n = chi - clo
            if off > 0:  # conv SAME pad: la[-1] = 0
                nc.sync.dma_start(la_c[r:r + 1, 0:off], zero_sb[:])
            if hi > chi:  # la[T] = 0
                nc.sync.dma_start(la_c[r:r + 1, TCH - (hi - chi):TCH],
                                  zero_sb[:])
            nc.sync.dma_start(la_c[r:r + 1, off:off + n],
                              la.ap()[b, clo:chi].unsqueeze(0))
        la_r = work.tile([6, TCH], F32R, tag="lar", name=f"lar{c}")
        nc.vector.tensor_copy(la_r[:], la_c[:])
        # 1) natural value tiles for this chunk (cast fp32 -> fp32r)
        vnat = []
        for i in range(NTS):
            vn = vnat_pool.tile([P, D], F32R, tag=f"vn{i}", name=f"vn{i}_{c}")
            nc.gpsimd.dma_start(
                vn[:],
                value.ap()[b, cc * TCH + i * P: cc * TCH + (i + 1) * P, :])
            vnat.append(vn)
        # 2) PE-transpose into vT k-slabs [128 d, TCH t]
        vts = []
        for k in range(NKT):
            vts.append(vt_pool.tile([P, TCH], F32R, tag=f"vt{k}",
                                    name=f"vt{k}_{c}"))
        for i in range(NTS):
            for k in range(NKT):
                pst = psum_tp.tile([P, P], F32, tag="tp",
                                   name=f"tp{c}_{i}_{k}")
                nc.tensor.transpose(
                    pst[:], vnat[i][:, k * P:(k + 1) * P].bitcast(F32),
                    ident[:])
                nc.vector.tensor_copy(vts[k][:, i * P:(i + 1) * P], pst[:])
        # 3) vproj + combo -> psum_h per a-tile; tanh; fc matmul
        pse = psum_sm.tile([1, TCH], F32, tag="e", name=f"pse{c}")
        for a in range(NAT):
            psh = psum_mm.tile([P, TCH], F32, tag="mm", name=f"psh{c}_{a}")
            for k in range(NKT):
                nc.tensor.matmul(
                    psh[:], wvt_sb[:, k, a * P:(a + 1) * P], vts[k][:],
                    start=(k == 0), stop=False)
            nc.tensor.matmul(
                psh[:], combo[:, b, a * P:(a + 1) * P],
                la_r[:], start=False, stop=True)
            hid = work.tile([P, TCH], F32, tag="hid", name=f"hid{c}_{a}")
            nc.scalar.activation(hid[:], psh[:], AF.Tanh)
            nc.tensor.matmul(
                pse[:], fcw_sb[:, a:a + 1], hid[:],
                start=(a == 0), stop=(a == NAT - 1))
        # 4) exp (+ chunk sum); fp32r copy feeds matmuls, fp32 copy for output
        aslice = attn_sb[:, b, cc * TCH:(cc + 1) * TCH]
        nc.scalar.activation(aslice, pse[:], AF.Exp,
                             accum_out=sums[:, b, cc:cc + 1])
        # 5) transpose exp row to partitions via K=1 matmuls
        p_sb = small.tile([P, NTS], F32, tag="p", name=f"p{c}")
        for i in range(NTS):
            psp = psum_pe.tile([P, 1], F32, tag="pe", name=f"tpe{c}_{i}")
            nc.tensor.matmul(psp[:], aslice[:, i * P:(i + 1) * P],
                             ones_f[:], start=True, stop=True)
            nc.vector.tensor_copy(p_sb[:, i:i + 1], psp[:])
        # 6) context accumulation (unnormalized exp weights)
        for i in range(NTS):
            for j in range(2):
                nc.tensor.matmul(
                    pctx[:, j, :], p_sb[:, i:i + 1],
                    vnat[i][:, j * TCH:(j + 1) * TCH].bitcast(F32),
                    start=(cc == 0 and i == 0),
                    stop=(cc == NCC - 1 and i == NTS - 1))
        # 7) per-batch epilogue
        if cc == NCC - 1:
            s_all = small.tile([1, 1], F32, tag="s", name=f"s{b}")
            nc.vector.reduce_sum(s_all[:], sums[:, b, :],
                                 axis=mybir.AxisListType.X)
            inv_s = small.tile([1, 1], F32, tag="is", name=f"is{b}")
            nc.vector.reciprocal(inv_s[:], s_all[:])
            ctx_sb = small.tile([1, D], F32, tag="ctx", name=f"ctxs{b}")
            nc.vector.tensor_scalar_mul(ctx_sb[:, 0:TCH], pctx[:, 0, :],
                                        inv_s[:])
            nc.vector.tensor_scalar_mul(ctx_sb[:, TCH:D], pctx[:, 1, :],
                                        inv_s[:])
            nc.sync.dma_start(ctx_out.ap()[b:b + 1, :], ctx_sb[:])
            nc.vector.tensor_scalar_mul(attn_sb[:, b, :], attn_sb[:, b, :],
                                        inv_s[:])
            nc.sync.dma_start(attn_out.ap()[b:b + 1, :], attn_sb[:, b, :])


_PROGRAM = None


def _get_program():
    global _PROGRAM
    if _PROGRAM is None:
        _PROGRAM = build_program()
    return _PROGRAM


def make_in_maps(query, value, last_attn, conv_w, conv_b, Wq, Wv, bias, fc_w):
    wvt_h = np.ascontiguousarray(Wv.T)
    wqt_h = np.ascontiguousarray(Wq.T)
    cw3_h = np.ascontiguousarray(conv_w[:, 0, :].T)
    cb_h = np.ascontiguousarray(conv_b.reshape(1, A))
    bias_h = np.ascontiguousarray(bias.reshape(1, A))
    fcw_h = np.ascontiguousarray(fc_w.reshape(A))
    in_maps = []
    for c in range(NCORES):
        sl = slice(c * BL, (c + 1) * BL)
        in_maps.append({
            "value": np.ascontiguousarray(value[sl]),
            "qt": np.ascontiguousarray(query[sl, 0, :].T),
            "la": np.ascontiguousarray(last_attn[sl]),
            "wvt": wvt_h, "wqt": wqt_h, "cw3": cw3_h, "cb": cb_h,
            "bias": bias_h, "fcw": fcw_h,
        })
    return in_maps


def kernel(query, value, last_attn, conv_w, conv_b, Wq, Wv, bias, fc_w, fc_b):
    asf = lambda x: np.asarray(x, dtype=np.float32)
    query, value, last_attn = asf(query), asf(value), asf(last_attn)
    conv_w, conv_b, Wq, Wv = asf(conv_w), asf(conv_b), asf(Wq), asf(Wv)
    bias, fc_w = asf(bias), asf(fc_w)

    nc = _get_program()
    in_maps = make_in_maps(query, value, last_attn, conv_w, conv_b, Wq, Wv,
                           bias, fc_w)
    res = run_bass_kernel_spmd(nc, in_maps, list(range(NCORES)))
    ctx = np.concatenate([res.results[c]["ctx_out"] for c in range(NCORES)],
                         axis=0)[:, None, :]
    attn = np.concatenate([res.results[c]["attn_out"] for c in range(NCORES)],
                          axis=0)
    return ctx.astype(np.float32), attn.astype(np.float32)
